# revision 37
# baseline (speedup 1.0000x reference)
"""Trainium2 Bass kernel for RoPE'd causal attention (no softmax).

Reference computation (B=2, H=8, T=2048, N=512, DV=128):
    QR = Q*cos + rotate_half_interleaved(Q)*sin         (K == Q)
    S  = QR @ QR^T          [B,H,T,T]
    S  = tril(S, -1)        (strictly lower triangular)
    O  = S @ V              [B,H,T,DV]

Sharding: the 16 (b,h) pairs are split 2-per-core across 8 NeuronCores.
Each core computes its two T x T score blocks independently; only the
strictly-lower-triangular block tiles are computed (upper tiles skipped),
and diagonal-straddling blocks only compute their live column range.

Device algorithm per (b,h):
  - RoPE on the vector engine (cos / sign-swapped-sin tables precomputed
    on host from the `freqs` input).
  - QR^T built via PE identity-transposes (fp32 has no DMA transpose).
  - Score blocks computed transposed (S^T[s,t]) so they feed the AV
    matmul as the moving operand with V as the stationary operand:
        pso[d, t-group] = sum_i V[i].T @ masked(S^T[i, t-group])
  - Output transposed back [d,t] -> [t,d] via PE and DMA'd out; the
    output block of group g is emitted after group g+1's matmuls so the
    PE never waits on the PSUM->SBUF copy.
"""

import math
import os

import numpy as np

B, H, T, NDIM, DV = 2, 8, 2048, 512, 128
P = 128            # partitions
NT = T // P        # 16 t-tiles per (b,h)
NG = 4             # t-groups per (b,h)
GW = T // NG       # 512 group width
NK = NDIM // P     # 4 contraction chunks
NCORES = 8
BH_PER_CORE = (B * H) // NCORES  # 2

# matmul input dtype: "f16" (1 cyc/row incl. 128-wide MMs + transposes),
# "f32r" (fast fp32: 1 cyc/row >=256 wide, 4 cyc/row at 128, 1.5 transpose)
# or "f32" (exact, 4 cyc/row)
MM_DT = os.environ.get("KERNEL_MM_DT", "f16")
# HAM warm-up filler matmuls sprinkled through the DMA/RoPE-bound ramp
WARM_N = int(os.environ.get("KERNEL_WARMN", "2"))

TRACE = False          # set by test harness to capture HW profile
LAST_RESULTS = None    # BassKernelResults of the last kernel() call

_NC_CACHE = {}


def _host_tables(freqs):
    """Mirror reference.py's fp32 phase arithmetic exactly."""
    f = np.asarray(freqs, dtype=np.float32).reshape(NDIM)
    t = np.arange(T, dtype=np.float32)
    ph = t[:, None] * f[None, :]            # fp32 multiply, like jnp
    ph = ph % np.float32(1.0)
    ph = ph * np.float32(2.0 * math.pi)
    cosv = np.cos(ph).astype(np.float32)
    sinv = np.sin(ph).astype(np.float32)
    # tmp = Q_pairswapped * ssw gives rotate_half(Q) * sin:
    #   ssw[t, 2i]   = -sin[t, 2i]
    #   ssw[t, 2i+1] = +sin[t, 2i+1]
    sign = np.tile(np.array([-1.0, 1.0], dtype=np.float32), NDIM // 2)
    ssw = sinv * sign[None, :]
    # fp16 halves the 8 MB of table DMA traffic that bounds the startup
    # ramp; cos/sin magnitudes are <= 1 so fp16's ~2^-11 absolute error is
    # below the fp32r matmul rounding already accepted.
    return cosv.astype(np.float16), np.ascontiguousarray(ssw).astype(np.float16)


def _host_masks():
    # mask_d[sp, tf] = 1.0 iff (128*i + sp) < (512*g + tf) with i = 4g + d
    sp = np.arange(P).reshape(P, 1)
    tf = np.arange(GW).reshape(1, GW)
    return np.stack(
        [(sp < (tf - P * d)) for d in range(NG)]
    ).astype(np.float32)


def _emit(tc, nc, aps):
    import concourse.mybir as mybir
    from contextlib import ExitStack
    from concourse.bass import ds, ts

    q, v, cosd, sswd, o = aps
    f32 = mybir.dt.float32
    # f16: everything PE-side is fp16 (1 cyc/row for all widths and for
    # transposes; PSUM accumulation stays fp32). f32r kept as fallback.
    if MM_DT == "f16":
        mmdt = mybir.dt.float16
    elif MM_DT == "f32r":
        mmdt = mybir.dt.float32r
    else:
        mmdt = f32

    def mmcast(ap):  # for DMA sources feeding f32r tiles (bit-identical)
        return ap.bitcast(mybir.dt.float32r) if MM_DT == "f32r" else ap

    with ExitStack() as ctx:

        def pool(name, bufs, space="SBUF"):
            return ctx.enter_context(
                tc.tile_pool(name=name, bufs=bufs, space=space)
            )

        const = pool("const", 1)
        cospool = pool("cost", NT)
        sswpool = pool("sswt", NT)
        qin = pool("qin", 3)
        qrp = pool("qr", 3)
        tmpp = pool("tmp", 3)
        qrtp = pool("qrt", 2 * NG)
        stp = pool("st", 2 * NG)  # group 0 keeps 4 tiles live per (b,h)
        vp = pool("v", 2)
        otp = pool("ot", 2)
        # each PSUM buf occupies at least a whole bank (8 banks total)
        ps_tr = pool("pstr", 2, "PSUM")
        ps_s = pool("pss", 2, "PSUM")
        ps_o = pool("pso", 2, "PSUM")
        ps_sc = pool("pssc", 1, "PSUM")   # group-0 score chunks (2 slots)
        ps_warm = pool("pswarm", 1, "PSUM")  # HAM warm-up filler target

        # Constants are built on the otherwise-idle GpSimd engine instead of
        # DMA'd, and table DMAs ride the scalar HWDGE ring while q tiles ride
        # the sync ring — the startup is DMA-bandwidth-bound, so every byte
        # and every serialized queue matters.
        def make_ident(name, dt_):
            t_ = const.tile([P, P], dt_, name=name)
            nc.gpsimd.memset(t_[:], 0.0)
            nc.gpsimd.affine_select(
                out=t_[:],
                in_=t_[:],
                compare_op=mybir.AluOpType.not_equal,
                fill=1.0,
                base=0,
                pattern=[[-1, P]],
                channel_multiplier=1,
            )
            return t_

        ident = make_ident("ident_f32", f32)
        if mmdt != f32:  # mmdt twin for all transposes (copy rounds the dtype)
            ident_r = const.tile([P, P], mmdt, name="ident_r")
            nc.scalar.copy(ident_r[:], ident[:])
        else:
            ident_r = ident

        # HAM warm-up fillers: the clock gate keeps the PE at 1.2 GHz until
        # it has been ~continuously busy for ~3.4 us, and transpose-mode MMs
        # do not count as PE-busy. During the DMA/RoPE-paced ramp the real
        # score-matmul diet is too sparse to unthrottle, so 512-wide filler
        # matmuls (on the already-resident cos table) are sprinkled between
        # ramp-phase ops. (A previous session measured warm-ups emitted
        # up-front as strictly worse; these are interleaved instead.)
        warm_ps = [None]

        def emit_warm(n):
            if mmdt != mybir.dt.float16:
                return
            for _ in range(n):
                if warm_ps[0] is None:
                    warm_ps[0] = ps_warm.tile([P, GW], f32, name="warm")
                nc.tensor.matmul(
                    warm_ps[0][:],
                    ident_r[:],
                    cos_t[0][:],
                    start=True,
                    stop=True,
                    skip_group_check=True,
                )

        mask_sb = const.tile([P, NG, GW], f32)
        for d in range(NG):
            # mask_d[sp, tf] = 1.0 iff sp < tf - 128*d
            nc.gpsimd.memset(mask_sb[:, d, :], 1.0)
            nc.gpsimd.affine_select(
                out=mask_sb[:, d, :],
                in_=mask_sb[:, d, :],
                compare_op=mybir.AluOpType.is_ge,
                fill=0.0,
                base=-(P * d + 1),
                pattern=[[1, GW]],
                channel_multiplier=-1,
            )
        cosr = cosd.rearrange("(j p) n -> j p n", p=P)
        sswr = sswd.rearrange("(j p) n -> j p n", p=P)
        cos_t = [None] * NT
        ssw_t = [None] * NT

        def load_tables(j):
            # Group-0 tables ride the scalar ring (parallel with q tiles on
            # the sync ring) to shorten the startup ramp; later tables go on
            # the sync ring so their dispatch cost doesn't serialize with the
            # scalar engine's PSUM->SBUF copies.
            eng = nc.scalar if j < NG else nc.sync
            ct = cospool.tile([P, NDIM], mybir.dt.float16)
            eng.dma_start(ct[:], cosr[j])
            st_ = sswpool.tile([P, NDIM], mybir.dt.float16)
            eng.dma_start(st_[:], sswr[j])
            cos_t[j] = ct
            ssw_t[j] = st_

        qr_ = q.rearrange("b (j p) n -> b j p n", p=P)    # [2,16,128,512]
        vr = v.rearrange("b (i s) d -> b s i d", s=P)     # [2,128,16,128]

        qdt = mmdt if MM_DT == "f16" else f32  # q arrives pre-cast in f16 mode

        def phase_a_tile(bh, j, qrt_g, jj):
            """DMA+RoPE one t-tile and transpose it into qrt_g."""
            if bh == 0:
                load_tables(j)
            qt = qin.tile([P, NDIM], qdt)
            nc.sync.dma_start(qt[:], qr_[bh, j])
            qr_tile = qrp.tile([P, NDIM], mmdt)
            tmp = tmpp.tile([P, NDIM], mmdt)
            nc.vector.tensor_mul(qr_tile[:], qt[:], cos_t[j][:])
            qsw = qt.rearrange("p (a two) -> p a two", two=2)[:, :, ::-1]
            nc.vector.tensor_tensor(
                tmp.rearrange("p (a two) -> p a two", two=2),
                qsw,
                ssw_t[j].rearrange("p (a two) -> p a two", two=2),
                mybir.AluOpType.mult,
            )
            nc.vector.tensor_add(qr_tile[:], qr_tile[:], tmp[:])
            # transpose output dtype must match its input dtype
            pst = ps_tr.tile([P, NK, P], mmdt)
            for nk in range(NK):
                nc.tensor.transpose(
                    pst[:, nk, :], qr_tile[:, ts(nk, P)], ident_r[:]
                )
            nc.scalar.copy(qrt_g[:, :, ts(jj, P)], pst[:])

        def group_output(bh, g, pso):
            """Stage the [d, t-group] PSUM block to SBUF and DMA to HBM.

            The device output stays transposed ([DV, T] per (b,h)); the host
            gather transposes back. This removes the PE output transposes
            and one of two PSUM->SBUF copies per group. (DMA cannot read
            PSUM directly, so one staging copy is unavoidable.)
            """
            ot = otp.tile([P, GW], f32)
            nc.scalar.copy(ot[:], pso[:])
            nc.sync.dma_start(o[bh, :, ds(g * GW, GW)], ot[:])

        # The two (b,h) of this core are interleaved group-by-group: phase A
        # of both, then phase B+C of both. This doubles the independent work
        # between pipeline boundaries so the PE never waits on the serial
        # DMA -> RoPE -> transpose chain of a single tile.
        v_sbs = [
            vp.tile([P, NT, DV], mmdt, name=f"v_sb{b_}")
            for b_ in range(BH_PER_CORE)
        ]
        qrt = [[] for _ in range(BH_PER_CORE)]  # [bh][g] QR^T group tiles
        pending = None  # (bh, g, pso) awaiting its deferred output block
        pending_av = None  # previous group's final AV matmul, deferred

        def emit_bc(bh, g):
            """Phase B+C: score blocks and AV accumulation for one group.

            Diagonal-straddling blocks (d = i - 4g >= 0) are zero for
            t-columns below lo = 128*d, so the score matmuls, the masked
            copy, and the AV matmul only touch the [lo:GW] column range.
            """
            nonlocal pending, pending_av
            v_sb = v_sbs[bh]
            qrt_g = qrt[bh][g]
            pso = ps_o.tile([P, GW], f32)
            ns = NG * g + NG  # number of s-tiles for this group
            av_args = []

            def emit_av(i):
                st_i, lo_i = av_args[i]
                nc.tensor.matmul(
                    pso[:, lo_i:],
                    v_sb[:, i, :],
                    st_i[:, lo_i:],
                    start=(i == 0),
                    stop=(i == ns - 1),
                    skip_group_check=True,
                )

            for i in range(ns):
                d = i - NG * g
                lo = P * d if d > 0 else 0
                pss = ps_s.tile([P, GW], f32)
                gi, ii = i // NG, i % NG
                for nk in range(NK):
                    nc.tensor.matmul(
                        pss[:, lo:],
                        qrt[bh][gi][:, nk, ts(ii, P)],
                        qrt_g[:, nk, lo:],
                        start=(nk == 0),
                        stop=(nk == NK - 1),
                        skip_group_check=True,
                    )
                st_t = stp.tile([P, GW], mmdt)
                if d >= 0:  # diagonal-straddling block: apply mask
                    nc.vector.tensor_tensor(
                        st_t[:, lo:],
                        pss[:, lo:],
                        mask_sb[:, d, lo:],
                        mybir.AluOpType.mult,
                    )
                else:
                    nc.scalar.copy(st_t[:], pss[:])
                av_args.append((st_t, lo))
                if i == 0 and pending_av is not None:
                    # previous group's final AV matmul, deferred past this
                    # group's first scores so its masked copy has finished
                    pending_av()
                    pending_av = None
                if i == 1 and pending is not None:
                    group_output(*pending)  # deferred previous-group output
                    pending = None
                if i > 0:  # AV matmul lags one step so the copy can finish
                    emit_av(i - 1)
            pending_av = lambda n_=ns - 1, f_=emit_av: f_(n_)  # noqa: E731
            pending = (bh, g, pso)

        # ---- group 0: tile-progressive scores across both (b,h) ----
        # 128-wide score chunks are emitted as soon as each RoPE'd t-tile
        # lands, so the PE gets real (HAM-visible) matmul work ~4 us earlier
        # and stays continuously busy through the DMA/RoPE-bound ramp.
        st_g0 = [[None] * NG for _ in range(BH_PER_CORE)]
        chunk_state = [None, 0]  # current [P,2,P] accumulator tile, slot idx

        def chunk_slot():
            s = chunk_state[1] % 2
            if s == 0:
                chunk_state[0] = ps_sc.tile([P, 2, P], f32, name="pssc")
            chunk_state[1] += 1
            return chunk_state[0], s

        pso_g0 = [None] * BH_PER_CORE

        def emit_scores_chunk(bh, j):
            """Score blocks (s-tile i <= j, t-chunk j) of group 0, then the
            AV accumulation for t-chunk j — all 128-wide, so the PE gets a
            dense diet of real (HAM-visible) matmuls through the ramp."""
            qrt_g = qrt[bh][0]
            for i in range(j + 1):
                pss, s = chunk_slot()
                for nk in range(NK):
                    nc.tensor.matmul(
                        pss[:, s, :],
                        qrt_g[:, nk, ts(i, P)],
                        qrt_g[:, nk, ts(j, P)],
                        start=(nk == 0),
                        stop=(nk == NK - 1),
                        skip_group_check=True,
                    )
                if i == j:  # diagonal chunk: strictly-lower mask
                    st_i = stp.tile([P, GW], mmdt, name="st_t")
                    st_g0[bh][i] = st_i
                    nc.vector.tensor_tensor(
                        st_i[:, ts(j, P)],
                        pss[:, s, :],
                        mask_sb[:, 0, 0:P],
                        mybir.AluOpType.mult,
                    )
                else:
                    nc.scalar.copy(st_g0[bh][i][:, ts(j, P)], pss[:, s, :])
            for i in range(j + 1):
                nc.tensor.matmul(
                    pso_g0[bh][:, ts(j, P)],
                    v_sbs[bh][:, i, :],
                    st_g0[bh][i][:, ts(j, P)],
                    start=(i == 0),
                    stop=(i == j),
                    skip_group_check=True,
                )

        for bh in range(BH_PER_CORE):
            qrt[bh].append(qrtp.tile([P, NK, GW], mmdt, name="qrt_g"))
            pso_g0[bh] = ps_o.tile([P, GW], f32, name="pso")
        for j in range(NG):
            for bh in range(BH_PER_CORE):
                phase_a_tile(bh, j, qrt[bh][0], j)
                emit_warm(WARM_N)
            if j == 0:
                # V s-tiles for the progressive group-0 AV chunks
                for bh in range(BH_PER_CORE):
                    nc.sync.dma_start(
                        v_sbs[bh][:, 0:NG, :], mmcast(vr[bh][:, 0:NG, :])
                    )
            for bh in range(BH_PER_CORE):
                emit_scores_chunk(bh, j)
        for bh in range(BH_PER_CORE):
            group_output(bh, 0, pso_g0[bh])

        # ---- groups 1..3: grouped 512-wide pipeline ----
        for g in range(1, NG):
            for bh in range(BH_PER_CORE):
                qrt_g = qrtp.tile([P, NK, GW], mmdt)
                qrt[bh].append(qrt_g)
                for jj in range(NG):
                    phase_a_tile(bh, NG * g + jj, qrt_g, jj)
                    if g == 1 and bh == 0:
                        # bridge the transpose-only stretch after the thin
                        # group 0 so HAM's idle window doesn't re-throttle
                        emit_warm(WARM_N)
                    if g == 1 and jj == 0 and bh == 0:
                        for b_ in range(BH_PER_CORE):
                            nc.sync.dma_start(
                                v_sbs[b_][:, NG:, :], mmcast(vr[b_][:, NG:, :])
                            )
            for bh in range(BH_PER_CORE):
                emit_bc(bh, g)
        pending_av()  # final group's last AV matmul
        group_output(*pending)  # final group's output


def build_nc():
    import concourse.bass as bass  # noqa: F401
    import concourse.mybir as mybir
    import concourse.tile as tile
    from concourse import bacc

    nc = bacc.Bacc(
        "TRN2",
        target_bir_lowering=False,
        debug=False,
        enable_asserts=False,
        num_devices=NCORES,
    )
    f32 = mybir.dt.float32
    f16 = mybir.dt.float16
    # In f16 mode q/v are pre-cast to fp16 on the host: halves the input DMA
    # (the startup ramp is DMA-bound) and enables packed 2x DVE modes.
    qvdt = f16 if MM_DT == "f16" else f32
    q = nc.dram_tensor("q", [BH_PER_CORE, T, NDIM], qvdt, kind="ExternalInput").ap()
    v = nc.dram_tensor("v", [BH_PER_CORE, T, DV], qvdt, kind="ExternalInput").ap()
    cosd = nc.dram_tensor("cosv", [T, NDIM], f16, kind="ExternalInput").ap()
    sswd = nc.dram_tensor("ssw", [T, NDIM], f16, kind="ExternalInput").ap()
    # output is written transposed ([DV, T] per (b,h)); host transposes back
    o = nc.dram_tensor("o", [BH_PER_CORE, DV, T], f32, kind="ExternalOutput").ap()

    with tile.TileContext(nc) as tc:
        _emit(tc, nc, (q, v, cosd, sswd, o))
    nc.compile()
    return nc


def get_nc():
    key = MM_DT
    if key not in _NC_CACHE:
        _NC_CACHE[key] = build_nc()
    return _NC_CACHE[key]


def make_in_maps(Q, V, freqs):
    qvdt = np.float16 if MM_DT == "f16" else np.float32
    Q = np.ascontiguousarray(np.asarray(Q, dtype=np.float32).reshape(B * H, T, NDIM).astype(qvdt))
    V = np.ascontiguousarray(np.asarray(V, dtype=np.float32).reshape(B * H, T, DV).astype(qvdt))
    cosv, ssw = _host_tables(freqs)
    in_maps = []
    for c in range(NCORES):
        in_maps.append(
            {
                "q": np.ascontiguousarray(Q[BH_PER_CORE * c : BH_PER_CORE * (c + 1)]),
                "v": np.ascontiguousarray(V[BH_PER_CORE * c : BH_PER_CORE * (c + 1)]),
                "cosv": cosv,
                "ssw": ssw,
            }
        )
    return in_maps


def kernel(Q, V, freqs):
    global LAST_RESULTS
    from concourse.bass_utils import run_bass_kernel_spmd

    nc = get_nc()
    in_maps = make_in_maps(Q, V, freqs)
    res = run_bass_kernel_spmd(
        nc, in_maps, core_ids=list(range(NCORES)), trace=TRACE
    )
    LAST_RESULTS = res
    out = np.stack([r["o"] for r in res.results])  # [8, 2, DV, T]
    out = np.ascontiguousarray(out.transpose(0, 1, 3, 2))  # -> [8, 2, T, DV]
    return out.reshape(B, H, T, DV).astype(np.float32)



# revision 40
# speedup vs baseline: 1.0493x; 1.0493x over previous
"""Trainium2 Bass kernel for RoPE'd causal attention (no softmax).

Reference computation (B=2, H=8, T=2048, N=512, DV=128):
    QR = Q*cos + rotate_half_interleaved(Q)*sin         (K == Q)
    S  = QR @ QR^T          [B,H,T,T]
    S  = tril(S, -1)        (strictly lower triangular)
    O  = S @ V              [B,H,T,DV]

Sharding: the 16 (b,h) pairs are split 2-per-core across 8 NeuronCores.
Each core computes its two T x T score blocks independently; only the
strictly-lower-triangular block tiles are computed (upper tiles skipped),
and diagonal-straddling blocks only compute their live column range.

Device algorithm per (b,h):
  - RoPE on the vector engine (cos / sign-swapped-sin tables precomputed
    on host from the `freqs` input).
  - QR^T built via PE identity-transposes (fp32 has no DMA transpose).
  - Score blocks computed transposed (S^T[s,t]) so they feed the AV
    matmul as the moving operand with V as the stationary operand:
        pso[d, t-group] = sum_i V[i].T @ masked(S^T[i, t-group])
  - Output transposed back [d,t] -> [t,d] via PE and DMA'd out; the
    output block of group g is emitted after group g+1's matmuls so the
    PE never waits on the PSUM->SBUF copy.
"""

import math
import os

import numpy as np

B, H, T, NDIM, DV = 2, 8, 2048, 512, 128
P = 128            # partitions
NT = T // P        # 16 t-tiles per (b,h)
NG = 4             # t-groups per (b,h)
GW = T // NG       # 512 group width
NK = NDIM // P     # 4 contraction chunks
NCORES = 8
BH_PER_CORE = (B * H) // NCORES  # 2

# matmul input dtype: "f16" (1 cyc/row incl. 128-wide MMs + transposes),
# "f32r" (fast fp32: 1 cyc/row >=256 wide, 4 cyc/row at 128, 1.5 transpose)
# or "f32" (exact, 4 cyc/row)
MM_DT = os.environ.get("KERNEL_MM_DT", "f16")
# HAM warm-up filler matmuls sprinkled through the DMA/RoPE-bound ramp
WARM_N = int(os.environ.get("KERNEL_WARMN", "2"))

TRACE = False          # set by test harness to capture HW profile
LAST_RESULTS = None    # BassKernelResults of the last kernel() call

_NC_CACHE = {}


def _host_tables(freqs):
    """Mirror reference.py's fp32 phase arithmetic exactly."""
    f = np.asarray(freqs, dtype=np.float32).reshape(NDIM)
    t = np.arange(T, dtype=np.float32)
    ph = t[:, None] * f[None, :]            # fp32 multiply, like jnp
    ph = ph % np.float32(1.0)
    ph = ph * np.float32(2.0 * math.pi)
    cosv = np.cos(ph).astype(np.float32)
    sinv = np.sin(ph).astype(np.float32)
    # tmp = Q_pairswapped * ssw gives rotate_half(Q) * sin:
    #   ssw[t, 2i]   = -sin[t, 2i]
    #   ssw[t, 2i+1] = +sin[t, 2i+1]
    sign = np.tile(np.array([-1.0, 1.0], dtype=np.float32), NDIM // 2)
    ssw = sinv * sign[None, :]
    # fp16 halves the 8 MB of table DMA traffic that bounds the startup
    # ramp; cos/sin magnitudes are <= 1 so fp16's ~2^-11 absolute error is
    # below the fp32r matmul rounding already accepted.
    return cosv.astype(np.float16), np.ascontiguousarray(ssw).astype(np.float16)


def _host_masks():
    # mask_d[sp, tf] = 1.0 iff (128*i + sp) < (512*g + tf) with i = 4g + d
    sp = np.arange(P).reshape(P, 1)
    tf = np.arange(GW).reshape(1, GW)
    return np.stack(
        [(sp < (tf - P * d)) for d in range(NG)]
    ).astype(np.float32)


def _emit(tc, nc, aps):
    import concourse.mybir as mybir
    from contextlib import ExitStack
    from concourse.bass import ds, ts

    q, v, cosd, sswd, o = aps
    f32 = mybir.dt.float32
    # f16: everything PE-side is fp16 (1 cyc/row for all widths and for
    # transposes; PSUM accumulation stays fp32). f32r kept as fallback.
    if MM_DT == "f16":
        mmdt = mybir.dt.float16
    elif MM_DT == "f32r":
        mmdt = mybir.dt.float32r
    else:
        mmdt = f32

    def mmcast(ap):  # for DMA sources feeding f32r tiles (bit-identical)
        return ap.bitcast(mybir.dt.float32r) if MM_DT == "f32r" else ap

    with ExitStack() as ctx:

        def pool(name, bufs, space="SBUF"):
            return ctx.enter_context(
                tc.tile_pool(name=name, bufs=bufs, space=space)
            )

        const = pool("const", 1)
        cospool = pool("cost", NT)
        sswpool = pool("sswt", NT)
        qin = pool("qin", 6)  # deep enough that qt DMA dispatches on the
        qrp = pool("qr", 3)   # sync queue don't block on buffer reuse
        tmpp = pool("tmp", 3)
        qrtp = pool("qrt", 2 * NG)
        stp = pool("st", 2 * NG)  # group 0 keeps 4 tiles live per (b,h)
        vp = pool("v", 2)
        otp = pool("ot", 2)
        # each PSUM buf occupies at least a whole bank (8 banks total)
        ps_tr = pool("pstr", 2, "PSUM")
        ps_s = pool("pss", 2, "PSUM")
        ps_o = pool("pso", 2, "PSUM")
        ps_sc = pool("pssc", 1, "PSUM")   # group-0 score chunks (2 slots)
        ps_warm = pool("pswarm", 1, "PSUM")  # HAM warm-up filler target

        # Constants are built on the otherwise-idle GpSimd engine instead of
        # DMA'd, and table DMAs ride the scalar HWDGE ring while q tiles ride
        # the sync ring — the startup is DMA-bandwidth-bound, so every byte
        # and every serialized queue matters.
        def make_ident(name, dt_):
            t_ = const.tile([P, P], dt_, name=name)
            nc.gpsimd.memset(t_[:], 0.0)
            nc.gpsimd.affine_select(
                out=t_[:],
                in_=t_[:],
                compare_op=mybir.AluOpType.not_equal,
                fill=1.0,
                base=0,
                pattern=[[-1, P]],
                channel_multiplier=1,
            )
            return t_

        ident = make_ident("ident_f32", f32)
        if mmdt != f32:  # mmdt twin for all transposes (copy rounds the dtype)
            ident_r = const.tile([P, P], mmdt, name="ident_r")
            nc.scalar.copy(ident_r[:], ident[:])
        else:
            ident_r = ident

        # HAM warm-up fillers: the clock gate keeps the PE at 1.2 GHz until
        # it has been ~continuously busy for ~3.4 us, and transpose-mode MMs
        # do not count as PE-busy. During the DMA/RoPE-paced ramp the real
        # score-matmul diet is too sparse to unthrottle, so 512-wide filler
        # matmuls (on the already-resident cos table) are sprinkled between
        # ramp-phase ops. (A previous session measured warm-ups emitted
        # up-front as strictly worse; these are interleaved instead.)
        warm_ps = [None]

        def emit_warm(n):
            if mmdt != mybir.dt.float16:
                return
            for _ in range(n):
                if warm_ps[0] is None:
                    warm_ps[0] = ps_warm.tile([P, GW], f32, name="warm")
                nc.tensor.matmul(
                    warm_ps[0][:],
                    ident_r[:],
                    cos_t[0][:],
                    start=True,
                    stop=True,
                    skip_group_check=True,
                )

        mask_sb = const.tile([P, NG, GW], f32)
        for d in range(NG):
            # mask_d[sp, tf] = 1.0 iff sp < tf - 128*d
            nc.gpsimd.memset(mask_sb[:, d, :], 1.0)
            nc.gpsimd.affine_select(
                out=mask_sb[:, d, :],
                in_=mask_sb[:, d, :],
                compare_op=mybir.AluOpType.is_ge,
                fill=0.0,
                base=-(P * d + 1),
                pattern=[[1, GW]],
                channel_multiplier=-1,
            )
        cosr = cosd.rearrange("(j p) n -> j p n", p=P)
        sswr = sswd.rearrange("(j p) n -> j p n", p=P)
        cos_t = [None] * NT
        ssw_t = [None] * NT

        def load_tables(j):
            # Group-0 tables ride the scalar ring (parallel with q tiles on
            # the sync ring) to shorten the startup ramp; later tables go on
            # the otherwise-idle gpsimd ring so neither the scalar engine's
            # PSUM->SBUF copies nor the sync ring's qt buffer-reuse waits
            # delay them (a blocked table DMA stalls vector RoPE, which
            # starves the PE at group boundaries).
            eng = nc.scalar if j < NG else nc.gpsimd
            ct = cospool.tile([P, NDIM], mybir.dt.float16)
            eng.dma_start(ct[:], cosr[j])
            st_ = sswpool.tile([P, NDIM], mybir.dt.float16)
            eng.dma_start(st_[:], sswr[j])
            cos_t[j] = ct
            ssw_t[j] = st_

        qr_ = q.rearrange("b (j p) n -> b j p n", p=P)    # [2,16,128,512]
        vr = v.rearrange("b (i s) d -> b s i d", s=P)     # [2,128,16,128]

        qdt = mmdt if MM_DT == "f16" else f32  # q arrives pre-cast in f16 mode

        def phase_a_tile(bh, j, qrt_g, jj):
            """DMA+RoPE one t-tile and transpose it into qrt_g."""
            if bh == 0:
                load_tables(j)
            qt = qin.tile([P, NDIM], qdt)
            nc.sync.dma_start(qt[:], qr_[bh, j])
            qr_tile = qrp.tile([P, NDIM], mmdt)
            tmp = tmpp.tile([P, NDIM], mmdt)
            nc.vector.tensor_mul(qr_tile[:], qt[:], cos_t[j][:])
            qsw = qt.rearrange("p (a two) -> p a two", two=2)[:, :, ::-1]
            nc.vector.tensor_tensor(
                tmp.rearrange("p (a two) -> p a two", two=2),
                qsw,
                ssw_t[j].rearrange("p (a two) -> p a two", two=2),
                mybir.AluOpType.mult,
            )
            nc.vector.tensor_add(qr_tile[:], qr_tile[:], tmp[:])
            # transpose output dtype must match its input dtype
            pst = ps_tr.tile([P, NK, P], mmdt)
            for nk in range(NK):
                nc.tensor.transpose(
                    pst[:, nk, :], qr_tile[:, ts(nk, P)], ident_r[:]
                )
            nc.scalar.copy(qrt_g[:, :, ts(jj, P)], pst[:])

        def group_output(bh, g, pso):
            """Stage the [d, t-group] PSUM block to SBUF and DMA to HBM.

            The device output stays transposed ([DV, T] per (b,h)); the host
            gather transposes back. This removes the PE output transposes
            and one of two PSUM->SBUF copies per group. (DMA cannot read
            PSUM directly, so one staging copy is unavoidable.)
            """
            ot = otp.tile([P, GW], f32)
            nc.scalar.copy(ot[:], pso[:])
            nc.gpsimd.dma_start(o[bh, :, ds(g * GW, GW)], ot[:])

        # The two (b,h) of this core are interleaved group-by-group: phase A
        # of both, then phase B+C of both. This doubles the independent work
        # between pipeline boundaries so the PE never waits on the serial
        # DMA -> RoPE -> transpose chain of a single tile.
        v_sbs = [
            vp.tile([P, NT, DV], mmdt, name=f"v_sb{b_}")
            for b_ in range(BH_PER_CORE)
        ]
        qrt = [[] for _ in range(BH_PER_CORE)]  # [bh][g] QR^T group tiles
        pending = None  # (bh, g, pso) awaiting its deferred output block
        pending_av = None  # previous group's final AV matmul, deferred

        def emit_bc(bh, g):
            """Phase B+C: score blocks and AV accumulation for one group.

            Diagonal-straddling blocks (d = i - 4g >= 0) are zero for
            t-columns below lo = 128*d, so the score matmuls, the masked
            copy, and the AV matmul only touch the [lo:GW] column range.
            """
            nonlocal pending, pending_av
            v_sb = v_sbs[bh]
            qrt_g = qrt[bh][g]
            pso = ps_o.tile([P, GW], f32)
            ns = NG * g + NG  # number of s-tiles for this group
            av_args = []

            def emit_av(i):
                st_i, lo_i = av_args[i]
                nc.tensor.matmul(
                    pso[:, lo_i:],
                    v_sb[:, i, :],
                    st_i[:, lo_i:],
                    start=(i == 0),
                    stop=(i == ns - 1),
                    skip_group_check=True,
                )

            for i in range(ns):
                d = i - NG * g
                lo = P * d if d > 0 else 0
                pss = ps_s.tile([P, GW], f32)
                gi, ii = i // NG, i % NG
                for nk in range(NK):
                    nc.tensor.matmul(
                        pss[:, lo:],
                        qrt[bh][gi][:, nk, ts(ii, P)],
                        qrt_g[:, nk, lo:],
                        start=(nk == 0),
                        stop=(nk == NK - 1),
                        skip_group_check=True,
                    )
                st_t = stp.tile([P, GW], mmdt)
                if d >= 0:  # diagonal-straddling block: apply mask
                    nc.vector.tensor_tensor(
                        st_t[:, lo:],
                        pss[:, lo:],
                        mask_sb[:, d, lo:],
                        mybir.AluOpType.mult,
                    )
                else:
                    nc.scalar.copy(st_t[:], pss[:])
                av_args.append((st_t, lo))
                if i == 0 and pending_av is not None:
                    # previous group's final AV matmul, deferred past this
                    # group's first scores so its masked copy has finished
                    pending_av()
                    pending_av = None
                if i == 1 and pending is not None:
                    group_output(*pending)  # deferred previous-group output
                    pending = None
                if i > 0:  # AV matmul lags one step so the copy can finish
                    emit_av(i - 1)
            pending_av = lambda n_=ns - 1, f_=emit_av: f_(n_)  # noqa: E731
            pending = (bh, g, pso)

        # ---- group 0: tile-progressive scores across both (b,h) ----
        # 128-wide score chunks are emitted as soon as each RoPE'd t-tile
        # lands, so the PE gets real (HAM-visible) matmul work ~4 us earlier
        # and stays continuously busy through the DMA/RoPE-bound ramp.
        st_g0 = [[None] * NG for _ in range(BH_PER_CORE)]
        chunk_state = [None, 0]  # current [P,2,P] accumulator tile, slot idx

        def chunk_slot():
            s = chunk_state[1] % 2
            if s == 0:
                chunk_state[0] = ps_sc.tile([P, 2, P], f32, name="pssc")
            chunk_state[1] += 1
            return chunk_state[0], s

        pso_g0 = [None] * BH_PER_CORE

        def emit_scores_chunk(bh, j):
            """Score blocks (s-tile i <= j, t-chunk j) of group 0, then the
            AV accumulation for t-chunk j — all 128-wide, so the PE gets a
            dense diet of real (HAM-visible) matmuls through the ramp."""
            qrt_g = qrt[bh][0]
            for i in range(j + 1):
                pss, s = chunk_slot()
                for nk in range(NK):
                    nc.tensor.matmul(
                        pss[:, s, :],
                        qrt_g[:, nk, ts(i, P)],
                        qrt_g[:, nk, ts(j, P)],
                        start=(nk == 0),
                        stop=(nk == NK - 1),
                        skip_group_check=True,
                    )
                if i == j:  # diagonal chunk: strictly-lower mask
                    st_i = stp.tile([P, GW], mmdt, name="st_t")
                    st_g0[bh][i] = st_i
                    nc.vector.tensor_tensor(
                        st_i[:, ts(j, P)],
                        pss[:, s, :],
                        mask_sb[:, 0, 0:P],
                        mybir.AluOpType.mult,
                    )
                else:
                    nc.scalar.copy(st_g0[bh][i][:, ts(j, P)], pss[:, s, :])
            for i in range(j + 1):
                nc.tensor.matmul(
                    pso_g0[bh][:, ts(j, P)],
                    v_sbs[bh][:, i, :],
                    st_g0[bh][i][:, ts(j, P)],
                    start=(i == 0),
                    stop=(i == j),
                    skip_group_check=True,
                )

        for bh in range(BH_PER_CORE):
            qrt[bh].append(qrtp.tile([P, NK, GW], mmdt, name="qrt_g"))
            pso_g0[bh] = ps_o.tile([P, GW], f32, name="pso")
        for j in range(NG):
            for bh in range(BH_PER_CORE):
                phase_a_tile(bh, j, qrt[bh][0], j)
                emit_warm(WARM_N)
            if j == 0:
                # V s-tiles for the progressive group-0 AV chunks
                for bh in range(BH_PER_CORE):
                    nc.gpsimd.dma_start(
                        v_sbs[bh][:, 0:NG, :], mmcast(vr[bh][:, 0:NG, :])
                    )
            for bh in range(BH_PER_CORE):
                emit_scores_chunk(bh, j)
        for bh in range(BH_PER_CORE):
            group_output(bh, 0, pso_g0[bh])

        # ---- groups 1..3: grouped 512-wide pipeline ----
        for g in range(1, NG):
            for bh in range(BH_PER_CORE):
                qrt_g = qrtp.tile([P, NK, GW], mmdt)
                qrt[bh].append(qrt_g)
                for jj in range(NG):
                    phase_a_tile(bh, NG * g + jj, qrt_g, jj)
                    if g == 1 and bh == 0:
                        # bridge the transpose-only stretch after the thin
                        # group 0 so HAM's idle window doesn't re-throttle
                        emit_warm(WARM_N)
                    if g == 1 and jj == 0 and bh == 0:
                        for b_ in range(BH_PER_CORE):
                            nc.gpsimd.dma_start(
                                v_sbs[b_][:, NG:, :], mmcast(vr[b_][:, NG:, :])
                            )
            for bh in range(BH_PER_CORE):
                emit_bc(bh, g)
        pending_av()  # final group's last AV matmul
        group_output(*pending)  # final group's output


def build_nc():
    import concourse.bass as bass  # noqa: F401
    import concourse.mybir as mybir
    import concourse.tile as tile
    from concourse import bacc

    nc = bacc.Bacc(
        "TRN2",
        target_bir_lowering=False,
        debug=False,
        enable_asserts=False,
        num_devices=NCORES,
    )
    f32 = mybir.dt.float32
    f16 = mybir.dt.float16
    # In f16 mode q/v are pre-cast to fp16 on the host: halves the input DMA
    # (the startup ramp is DMA-bound) and enables packed 2x DVE modes.
    qvdt = f16 if MM_DT == "f16" else f32
    q = nc.dram_tensor("q", [BH_PER_CORE, T, NDIM], qvdt, kind="ExternalInput").ap()
    v = nc.dram_tensor("v", [BH_PER_CORE, T, DV], qvdt, kind="ExternalInput").ap()
    cosd = nc.dram_tensor("cosv", [T, NDIM], f16, kind="ExternalInput").ap()
    sswd = nc.dram_tensor("ssw", [T, NDIM], f16, kind="ExternalInput").ap()
    # output is written transposed ([DV, T] per (b,h)); host transposes back
    o = nc.dram_tensor("o", [BH_PER_CORE, DV, T], f32, kind="ExternalOutput").ap()

    with tile.TileContext(nc) as tc:
        _emit(tc, nc, (q, v, cosd, sswd, o))
    nc.compile()
    return nc


def get_nc():
    key = MM_DT
    if key not in _NC_CACHE:
        _NC_CACHE[key] = build_nc()
    return _NC_CACHE[key]


def make_in_maps(Q, V, freqs):
    qvdt = np.float16 if MM_DT == "f16" else np.float32
    Q = np.ascontiguousarray(np.asarray(Q, dtype=np.float32).reshape(B * H, T, NDIM).astype(qvdt))
    V = np.ascontiguousarray(np.asarray(V, dtype=np.float32).reshape(B * H, T, DV).astype(qvdt))
    cosv, ssw = _host_tables(freqs)
    in_maps = []
    for c in range(NCORES):
        in_maps.append(
            {
                "q": np.ascontiguousarray(Q[BH_PER_CORE * c : BH_PER_CORE * (c + 1)]),
                "v": np.ascontiguousarray(V[BH_PER_CORE * c : BH_PER_CORE * (c + 1)]),
                "cosv": cosv,
                "ssw": ssw,
            }
        )
    return in_maps


def kernel(Q, V, freqs):
    global LAST_RESULTS
    from concourse.bass_utils import run_bass_kernel_spmd

    nc = get_nc()
    in_maps = make_in_maps(Q, V, freqs)
    res = run_bass_kernel_spmd(
        nc, in_maps, core_ids=list(range(NCORES)), trace=TRACE
    )
    LAST_RESULTS = res
    out = np.stack([r["o"] for r in res.results])  # [8, 2, DV, T]
    out = np.ascontiguousarray(out.transpose(0, 1, 3, 2))  # -> [8, 2, T, DV]
    return out.reshape(B, H, T, DV).astype(np.float32)

